# revision 7
# baseline (speedup 1.0000x reference)
"""DiagonalLinear kernel for Trainium2: y = x * diagonal (broadcast over last axis).

Full input x is [32768, 4096] f32, diagonal is [4096] f32. Data-parallel over
8 NeuronCores: each core owns a [4096, 4096] row-shard of x; the diagonal is
replicated. Per core the shard is streamed through SBUF in [128, K*4096]
tiles (K consecutive rows per partition, contiguous in DRAM), multiplied on
the vector engine against a [128, 4096] SBUF copy of the diagonal
(broadcast across partitions once via a stride-0 DMA), and streamed back.
"""

import numpy as np

N_ROWS = 32768
CHANNELS = 4096
N_CORES = 8
ROWS_PER_CORE = N_ROWS // N_CORES  # 4096
P = 128

# K = consecutive rows packed into one partition's free dim. A tile is
# [128, K*CHANNELS] f32 = K*2 MiB, DMA'd as one contiguous run per partition.
K = 2
ROWS_PER_TILE = P * K  # 256
N_TILES = ROWS_PER_CORE // ROWS_PER_TILE  # 16
BUFS = 4

_NC_CACHE = {}


def _build_nc(k=K, bufs=BUFS, store_on_act=True):
    import concourse.bass as bass
    import concourse.bacc as bacc
    import concourse.mybir as mybir
    from concourse.tile import TileContext

    # Bacc (not raw Bass): its compile() pass splits multi-sem waits into
    # EventSemaphore chains — TRN2 allows at most 1 sync wait per instruction.
    nc = bacc.Bacc("TRN2", target_bir_lowering=False, debug=False)
    x_t = nc.dram_tensor(
        "x", [ROWS_PER_CORE, CHANNELS], mybir.dt.float32, kind="ExternalInput"
    )
    d_t = nc.dram_tensor(
        "diagonal", [CHANNELS], mybir.dt.float32, kind="ExternalInput"
    )
    o_t = nc.dram_tensor(
        "out", [ROWS_PER_CORE, CHANNELS], mybir.dt.float32, kind="ExternalOutput"
    )

    rows_per_tile = P * k
    n_tiles = ROWS_PER_CORE // rows_per_tile
    x_ap = x_t.ap()
    o_ap = o_t.ap()
    d_ap = d_t.ap()

    with TileContext(nc) as tc:
        with (
            tc.tile_pool(name="singles", bufs=1) as singles,
            tc.tile_pool(name="work", bufs=bufs) as work,
        ):
            # Diagonal broadcast across all 128 partitions: stride-0 DMA on
            # the partition dim. Issued on gpsimd (SWDGE) so it doesn't
            # occupy the HWDGE rings that stream x.
            diag_tile = singles.tile([P, CHANNELS], mybir.dt.float32)
            diag_bcast = bass.AP(
                tensor=d_ap.tensor,
                offset=d_ap.offset,
                ap=[[0, P], list(d_ap.ap[0])],
            )
            nc.gpsimd.dma_start(out=diag_tile[:], in_=diag_bcast)
            # Pre-consume diag_tile on DVE: the TensorTensor ISA struct has a
            # single sync-wait slot, so the first mul must not need waits on
            # both the diag DMA and its x-load DMA. This copy absorbs the
            # diag-DMA wait; later DVE ops inherit it via the vector clock.
            scratch = singles.tile([P, 1], mybir.dt.float32)
            nc.vector.tensor_copy(scratch[:], diag_tile[:, :1])

            store_engine = nc.scalar if store_on_act else nc.sync

            for i in range(n_tiles):
                t = work.tile([P, k, CHANNELS], mybir.dt.float32)
                # Rows [i*rows_per_tile, (i+1)*rows_per_tile): partition p
                # holds rows i*rows_per_tile + p*k .. +k-1, contiguous.
                src = x_ap[i * rows_per_tile : (i + 1) * rows_per_tile, :].rearrange(
                    "(p k) c -> p k c", p=P
                )
                dst = o_ap[i * rows_per_tile : (i + 1) * rows_per_tile, :].rearrange(
                    "(p k) c -> p k c", p=P
                )
                nc.sync.dma_start(out=t[:], in_=src)
                nc.vector.tensor_mul(
                    t[:], t[:], diag_tile[:, None, :].to_broadcast((P, k, CHANNELS))
                )
                store_engine.dma_start(out=dst, in_=t[:])

    # Bacc defers register allocation / wait splitting to compile(), which
    # finalize() runs; run_bass_kernel_spmd expects a finalized module.
    nc.finalize()
    return nc


def _get_nc(**kwargs):
    key = tuple(sorted(kwargs.items()))
    if key not in _NC_CACHE:
        _NC_CACHE[key] = _build_nc(**kwargs)
    return _NC_CACHE[key]


def _enable_tracing():
    """Make trace=True work in this container: register the NTFF profile
    hook (the image's antenv stub lacks axon_hooks) and keep trace
    artifacts local instead of uploading."""
    import sys
    import types

    if "antenv.axon_hooks" not in sys.modules:
        from trn_agent_boot.trn_boot import _ntff_profile_via_ctypes

        hook = _ntff_profile_via_ctypes("/opt/axon/libaxon_pjrt.so")
        mod = types.ModuleType("antenv.axon_hooks")
        mod.get_axon_ntff_profile_hook = lambda: hook
        mod.set_axon_ntff_profile_hook = lambda h: None
        sys.modules["antenv.axon_hooks"] = mod

    from concourse import bass_utils

    bass_utils.upload_artifacts = lambda tmpdir: tmpdir


def run(x, diagonal, trace=False, trace_cores=None, tmpdir=None, **build_kwargs):
    """Shard, run on 8 cores, gather. Returns (out, BassKernelResults)."""
    from concourse.bass_utils import run_bass_kernel_spmd

    if trace:
        _enable_tracing()

    x = np.ascontiguousarray(x, dtype=np.float32)
    diagonal = np.ascontiguousarray(diagonal, dtype=np.float32)
    assert x.shape == (N_ROWS, CHANNELS), x.shape
    assert diagonal.shape == (CHANNELS,), diagonal.shape

    nc = _get_nc(**build_kwargs)
    in_maps = [
        {"x": x[i * ROWS_PER_CORE : (i + 1) * ROWS_PER_CORE], "diagonal": diagonal}
        for i in range(N_CORES)
    ]
    res = run_bass_kernel_spmd(
        nc,
        in_maps,
        list(range(N_CORES)),
        trace=trace,
        trace_cores=trace_cores,
        tmpdir=tmpdir,
    )
    out = np.concatenate([res.results[i]["out"] for i in range(N_CORES)], axis=0)
    return out, res


def kernel(x, diagonal):
    out, _ = run(x, diagonal)
    return out
